# revision 19
# baseline (speedup 1.0000x reference)
"""Trainium2 Bass kernel for nn_AttentionDecoder (N=100000, H=256, 8 cores).

Math reduction
--------------
With W_ks = W_static_kvl[:, :H], W_vs = W_static_kvl[:, H:2H] (same split for
W_dyn_kvl), the reference collapses to one pass over the only large tensors
(h_static, h_dynamic):

    compat   = h_s @ u_s + h_d @ u_d        with u_* = (W_k* @ q)/sqrt(H)
    p_i      = exp(compat_i - SHIFT)        (valid nodes only)
    context  = ((p @ h_s) @ W_vs + (p @ h_d) @ W_vd) / sum(p)

Invalid nodes (valid_mask=False) get alpha = 0 exactly in the reference
(exp(-1e9 - max) == 0 in fp32), so they contribute nothing to any sum.  The
host therefore COMPACTS to the valid rows before sharding: with the ~50%
Bernoulli mask this halves DMA traffic, halves the weighted-sum matmuls and
halves the compat work.  Pad rows are zero, each contributing exactly
exp(-SHIFT) to s (and 0 to t); the host subtracts pad_count*exp(-SHIFT).

Device kernel (per core, TILES node-tiles of 128 x [h_s|h_d] fp16):
  compat per tile via one of three engine-balanced paths:
    * A: fused VectorE multiply+row-reduce (scalar_tensor_tensor + accum);
    * B: VectorE multiply at 2x fp16, ScalarE Identity-activation accumulate;
    * P: TensorE contracts a host-shipped transposed copy against u into a
      PSUM column (4 chunk matmuls).
  All of a block's compat values are exponentiated by at most two BATCHED
  ScalarE Exp instructions (bias = -SHIFT constant; the ACT fixed cost of
  ~352 cycles/instruction makes per-tile exps prohibitive).
  t += p-weighted row sums: TensorE matmul (lhsT = p column, rhs = tile),
  rotated over 4 PSUM banks, deferred one block so the PE never stalls.
  DMA: block sizes ramp 2,3,5,7,8,... so compute starts ~9us into the NEFF
  instead of ~18 (the SP preamble + first-transfer latency is the floor).
Host gathers per-core partials (t rows, s column) and runs the tiny MLP
head + exact jax sampling.
"""

import math

import numpy as np

import concourse.bacc as bacc
import concourse.mybir as mybir
import concourse.tile as tile
from concourse import bass_utils

# ---- problem constants (hardcoded per harness contract) ----
H = 256
NCORES = 8
P = 128                     # SBUF partitions
NBANK = 3                   # PSUM col-groups rotated for the weighted-sum matmuls
BMAX = 8                    # max tiles per DMA block
NBCAP = 3                   # max B tiles per block (ub3 width)
CPMAX = 8                   # max PE-path tiles per block (one PSUM tile wide)
SHIFT = 8.0
NEG = np.float32(-1e9)

# test.py hooks
TRACE_OPTS: dict = {}
LAST_RESULTS = None
LAST_INTERNALS: dict = {}

_prog_cache: dict = {}


def _make_plan(tiles):
    """Static schedule for a per-core tile count.

    Returns dict with:
      sizes:   list of block sizes (sum == tiles)
      paths:   per block (nA, nB, nP) with nA+nB+nP == size
      p_tiles: global list of tile indices served by the PE path (their
               serial order == layout order of the transposed pack)
      nb_max:  max nB over blocks (width of the ub3 broadcast tensor)
    """
    sizes = []
    rem = tiles
    for r in (1, 2, 4, 7):
        if rem <= 0:
            break
        s = min(r, rem)
        sizes.append(s)
        rem -= s
    while rem > 0:
        s = min(BMAX, rem)
        sizes.append(s)
        rem -= s
    # keep the tail blocks small so the post-DMA drain is short
    if len(sizes) >= 2 and sizes[-1] > 4:
        s = sizes[-1]
        sizes[-1] = 4
        sizes.append(s - 4)
    if len(sizes) >= 2 and sizes[-1] > 2:
        s = sizes[-1]
        sizes[-1] = s - 2
        sizes.append(2)

    nblk = len(sizes)
    # global path targets (empirically engine-balanced; see module docstring)
    n_p = int(round(0.33 * tiles))
    n_b = int(round(0.33 * tiles))
    if tiles < 6:
        n_p = 0
        n_b = 0

    # spread P over blocks 2..nblk-2 (u/tb singles load during 0-1; keep the
    # tail block DVE/ACT-only so the drain is short), cap CPMAX per block
    nP = [0] * nblk
    elig_p = list(range(2, nblk - 1))
    k = 0
    while k < n_p and elig_p:
        done = True
        for b in elig_p:
            if k >= n_p:
                break
            if nP[b] < min(3, CPMAX, sizes[b] - 1):
                nP[b] += 1
                k += 1
                done = False
        if done:
            break
    n_p = k
    # spread B evenly from block 1 on (ACT would otherwise idle early,
    # then become the post-DMA drain bottleneck)
    nB = [0] * nblk
    elig_b = list(range(1, nblk))
    k = 0
    while k < n_b and elig_b:
        done = True
        for b in elig_b:
            if k >= n_b:
                break
            if nB[b] < min(NBCAP, sizes[b] - nP[b] - (1 if b < nblk - 1 else 0)):
                nB[b] += 1
                k += 1
                done = False
        if done:
            break
    n_b = k

    paths = []
    p_tiles = []
    t0 = 0
    for b in range(nblk):
        na = sizes[b] - nB[b] - nP[b]
        assert na >= 0
        paths.append((na, nB[b], nP[b]))
        for j in range(nP[b]):
            p_tiles.append(t0 + na + nB[b] + j)
        t0 += sizes[b]
    nb_max = max(nB) if nB else 0
    return dict(sizes=sizes, paths=paths, p_tiles=p_tiles,
                nb_max=max(min(nb_max, NBCAP), 1), npt=len(p_tiles))


def _build_program(tiles):
    key = ("v23", tiles)
    if key in _prog_cache:
        return _prog_cache[key]

    plan = _make_plan(tiles)
    sizes, paths = plan["sizes"], plan["paths"]
    nb_max, npt = plan["nb_max"], plan["npt"]
    npad = P * tiles

    f32 = mybir.dt.float32
    f16 = mybir.dt.float16
    nc = bacc.Bacc(
        "TRN2",
        target_bir_lowering=False,
        debug=False,
        enable_asserts=False,
        num_devices=NCORES,
    )
    hh = nc.dram_tensor("hh", [npad, 2 * H], f16, kind="ExternalInput").ap()
    ubx = nc.dram_tensor("ubx", [P, 2 * H + 8], f16, kind="ExternalInput").ap()
    ub3 = nc.dram_tensor(
        "ub3", [P, nb_max, 2 * H], f16, kind="ExternalInput"
    ).ap()
    nhb = max(npt, 1)
    hb = nc.dram_tensor("hb", [2 * H, nhb * P], f16, kind="ExternalInput").ap()
    t_out = nc.dram_tensor("t_out", [NBANK, 2 * H + 1], f32,
                           kind="ExternalOutput").ap()

    hh_g = hh.rearrange("(p t) h -> p t h", t=tiles)
    hb_g = hb.rearrange("(c p) n -> p c n", p=P)
    nbank = min(NBANK, tiles)

    with tile.TileContext(nc) as tc:
        with (
            tc.tile_pool(name="singles", bufs=1) as singles,
            tc.tile_pool(name="blocks", bufs=5) as blocks,
            tc.tile_pool(name="small", bufs=4) as small,
            tc.tile_pool(name="scratch", bufs=4) as scratch,
            tc.tile_pool(name="psum", bufs=1, space="PSUM") as psum,
        ):
            p_grid = singles.tile([P, tiles], f16)
            ubx_sb = singles.tile([P, 2 * H + 8], f16)
            u_sb = ubx_sb[:, 0:2 * H]
            uc_sb = ubx_sb[:, 2 * H:2 * H + 4]
            nshift_sb = ubx_sb[:, 2 * H + 4:2 * H + 5]
            ones_sb = ubx_sb[:, 2 * H + 5:2 * H + 6]
            u3_sb = singles.tile([P, nb_max, 2 * H], f16)

            t_all = psum.tile([P, 2 * H], f32, tag="tall")
            c_ps = []
            for j in range(2):
                cpsj = psum.tile([P, CPMAX], f32, tag=f"cps{j}")
                c_ps.append(cpsj)
            s_ps = psum.tile([1, tiles], f32, tag="sps")

            nblk = len(sizes)
            starts = [sum(sizes[:i]) for i in range(nblk)]
            pending = []
            exp_q = []   # (t0, sz, nAB, nP, cp, cblk) awaiting the exp stage
            kP = 0  # global PE-tile serial
            for b in range(nblk + 1):
                if b < nblk:
                    sz = sizes[b]
                    t0 = starts[b]
                    nA, nB, nP = paths[b]
                    nAB = nA + nB
                    buf = blocks.tile([P, BMAX, 2 * H], f16)
                    dma_eng = nc.scalar if b < 2 else nc.sync
                    dma_eng.dma_start(
                        out=buf[:, 0:sz, :], in_=hh_g[:, t0:t0 + sz, :]
                    )
                    if b == 0:
                        nc.sync.dma_start(out=ubx_sb, in_=ubx)
                    elif b == 1:
                        nc.sync.dma_start(out=u3_sb, in_=ub3)
                    if nP > 0:
                        tb = blocks.tile([P, 4, CPMAX * P], f16, tag="tb")
                        nc.sync.dma_start(
                            out=tb[:, :, 0:nP * P],
                            in_=hb_g[:, :, kP * P:(kP + nP) * P],
                        )

                    # P path: PE contracts transposed tiles against u, one
                    # PSUM column per tile
                    cp = c_ps[b % 2]
                    for j in range(nP):
                        for ch in range(4):
                            nc.tensor.matmul(
                                cp[:, j:j + 1],
                                lhsT=tb[:, ch, j * P:(j + 1) * P],
                                rhs=uc_sb[:, ch:ch + 1],
                                start=(ch == 0),
                                stop=(ch == 3),
                            )

                    cblk = small.tile([P, BMAX], f32)
                    # B path: one wide 2x fp16 multiply for the B tiles
                    if nB > 0:
                        scv = scratch.tile([P, nb_max, 2 * H], f16, tag="bmul")
                        nc.vector.tensor_mul(
                            scv[:, 0:nB, :], buf[:, nA:nA + nB, :],
                            u3_sb[:, 0:nB, :]
                        )
                    # A path: fused DVE multiply + row-reduce
                    for g in range(nA):
                        sc = scratch.tile([P, 2 * H], f16, tag="sttout")
                        nc.vector.scalar_tensor_tensor(
                            out=sc,
                            in0=buf[:, g, :],
                            scalar=1.0,
                            in1=u_sb,
                            op0=mybir.AluOpType.mult,
                            op1=mybir.AluOpType.mult,
                            accum_out=cblk[:, g:g + 1],
                        )
                    # B path: ScalarE accumulates each tile's row
                    for j in range(nB):
                        sc2 = scratch.tile([P, 2 * H], f16, tag="actout")
                        nc.scalar.activation(
                            out=sc2,
                            in_=scv[:, j, :],
                            func=mybir.ActivationFunctionType.Identity,
                            bias=0.0,
                            scale=1.0,
                            accum_out=cblk[:, nA + j:nA + j + 1],
                        )
                    kP += nP
                    exp_q.append((t0, sz, nAB, nP, cp, cblk))
                    pending.append((t0, sz, buf))

                # exp stage, one block behind: the tiny PSUM->SBUF copy is
                # emitted on DVE only after the NEXT block's multiplies, so
                # it never head-of-line-blocks DVE on the PE compat matmuls
                if len(exp_q) > (1 if b < nblk else 0):
                    et0, esz, enAB, enP, ecp, ecblk = exp_q.pop(0)
                    if enP > 0:
                        nc.vector.tensor_copy(
                            ecblk[:, enAB:esz], ecp[:, 0:enP]
                        )
                    nc.scalar.activation(
                        out=p_grid[:, et0:et0 + esz],
                        in_=ecblk[:, 0:esz],
                        func=mybir.ActivationFunctionType.Exp,
                        bias=nshift_sb,
                        scale=1.0,
                    )

                # weighted sums, two blocks behind
                while len(pending) > (2 if b < nblk else 0):
                    pt0, psz, pbuf = pending.pop(0)
                    for g in range(psz):
                        c = pt0 + g
                        r = 32 * (c % nbank)
                        nc.tensor.matmul(
                            t_all[r:r + 1, :],
                            lhsT=p_grid[:, c:c + 1],
                            rhs=pbuf[:, g, :],
                            start=(c < nbank),
                            stop=(c >= tiles - nbank),
                        )

            # s = sum(p): partition-reduce via ones-matmul, then a tiny
            # free-dim reduce on the [1, tiles] PSUM row
            nc.tensor.matmul(s_ps, lhsT=ones_sb, rhs=p_grid, start=True,
                             stop=True)
            # one copy moves all four bank rows (partitions 0/32/64/96)
            t_sb = small.tile([P, 2 * H + 1], f32, tag="tsb")
            if nbank < NBANK:
                nc.scalar.copy(t_sb[:, 0:2 * H], t_all)
            nc.vector.tensor_copy(t_sb[:, 0:2 * H], t_all)
            nc.vector.reduce_sum(out=t_sb[0:1, 2 * H:], in_=s_ps,
                                 axis=mybir.AxisListType.X)
            nc.sync.dma_start(
                out=t_out, in_=t_sb[0:NBANK * 32:32, :]
            )

    nc.compile()
    _prog_cache[key] = (nc, plan)
    return nc, plan


def _run_device(h_static, h_dynamic, u_cat, valid_idx):
    """Stream the compacted valid rows through the 8-core SPMD kernel.

    Returns (t [2H] float64 summed over cores, s float64, pad-corrected).
    """
    global LAST_RESULTS

    nv = len(valid_idx)
    q = (nv + NCORES - 1) // NCORES
    tiles = max(1, (q + P - 1) // P)
    npad = P * tiles
    nc, plan = _build_program(tiles)
    npt, nb_max = plan["npt"], plan["nb_max"]
    p_tiles = plan["p_tiles"]

    u16 = u_cat.astype(np.float16)
    ubx = np.zeros((P, 2 * H + 8), np.float16)
    ubx[:, 0:2 * H] = u16
    ubx[:, 2 * H:2 * H + 4] = u16.reshape(4, P).T
    ubx[:, 2 * H + 4] = np.float16(-SHIFT)
    ubx[:, 2 * H + 5] = np.float16(1.0)
    ub3 = np.ascontiguousarray(np.broadcast_to(u16, (P, nb_max, 2 * H)))

    total_pad = 0
    in_maps = []
    for c in range(NCORES):
        rows = valid_idx[c * q:(c + 1) * q]
        nr = len(rows)
        total_pad += npad - nr
        h16 = np.zeros((npad, 2 * H), np.float16)
        if nr:
            h16[:nr, 0:H] = h_static[rows]
            h16[:nr, H:2 * H] = h_dynamic[rows]
        nhb = max(npt, 1)
        hbt = np.zeros((2 * H, nhb * P), np.float16)
        for k, t in enumerate(p_tiles):
            hbt[:, k * P:(k + 1) * P] = h16[t::tiles, :].T
        in_maps.append({"hh": h16, "ubx": ubx, "ub3": ub3,
                        "hb": np.ascontiguousarray(hbt)})

    res = bass_utils.run_bass_kernel_spmd(
        nc, in_maps, core_ids=list(range(NCORES)), **TRACE_OPTS
    )
    LAST_RESULTS = res

    t = np.zeros(2 * H, np.float64)
    s = 0.0
    for c in range(NCORES):
        arr = res.results[c]["t_out"].astype(np.float64)
        t += arr[:, :2 * H].sum(axis=0)
        s += arr[0, 2 * H]
    s -= total_pad * math.exp(-SHIFT)
    return t, s


def kernel(
    h_dynamic,
    h_static,
    W_static_kvl,
    W_dyn_kvl,
    W_q,
    W1,
    b1,
    W2,
    b2,
    valid_mask,
    current_node,
):
    h_dynamic = np.asarray(h_dynamic, np.float32)
    h_static = np.asarray(h_static, np.float32)
    W_static_kvl = np.asarray(W_static_kvl, np.float32)
    W_dyn_kvl = np.asarray(W_dyn_kvl, np.float32)
    W_q = np.asarray(W_q, np.float32)
    W1 = np.asarray(W1, np.float32)
    b1 = np.asarray(b1, np.float32)
    W2 = np.asarray(W2, np.float32)
    b2 = np.asarray(b2, np.float32)
    valid = np.asarray(valid_mask).astype(bool)
    cur = int(current_node)

    scale = 1.0 / math.sqrt(H)

    # ---- tiny host-side prologue (exact math on one row) ----
    h_cur = (h_static[cur].astype(np.float64) + h_dynamic[cur].astype(np.float64))
    q = h_cur @ W_q.astype(np.float64)  # [H]
    u_s = (W_static_kvl[:, 0:H].astype(np.float64) @ q) * scale
    u_d = (W_dyn_kvl[:, 0:H].astype(np.float64) @ q) * scale
    u_cat = np.concatenate([u_s, u_d]).astype(np.float32)  # [2H]

    valid_idx = np.flatnonzero(valid)

    W_vs = W_static_kvl[:, H:2 * H].astype(np.float64)
    W_vd = W_dyn_kvl[:, H:2 * H].astype(np.float64)

    if len(valid_idx) == 0:
        # all-masked edge case: reference softmax degenerates to uniform
        # over all N nodes; context is the mean of V. The logit cancels in
        # the final output anyway; run the device on a dummy row for timing.
        t, s = _run_device(h_static, h_dynamic, u_cat, np.array([0]))
        n = h_static.shape[0]
        context = (h_static.mean(0).astype(np.float64) @ W_vs
                   + h_dynamic.mean(0).astype(np.float64) @ W_vd)
    else:
        t, s = _run_device(h_static, h_dynamic, u_cat, valid_idx)
        context = (t[:H] @ W_vs + t[H:] @ W_vd) / s  # [H]

    # ---- tiny host-side epilogue ----
    fuse = np.concatenate([h_cur, context])  # [2H]
    hidden = np.maximum(fuse @ W1.astype(np.float64) + b1.astype(np.float64), 0.0)
    logit = float(hidden @ W2.astype(np.float64)[:, 0] + float(b2[0]))

    logits_all = np.where(valid, np.float32(logit), NEG).astype(np.float32)

    LAST_INTERNALS.update(
        dict(u_cat=u_cat, t=t, s=s, context=context, logit=logit)
    )

    # exact replication of the reference's sampling (jax threefry, key(1))
    import contextlib

    import jax
    import jax.numpy as jnp

    try:
        ctx = jax.default_device(jax.devices("cpu")[0])
    except Exception:
        ctx = contextlib.nullcontext()
    with ctx:
        logits_j = jnp.asarray(logits_all)
        choice = jax.random.categorical(jax.random.key(1), logits_j)
        log_probs = jax.nn.log_softmax(logits_j)
        log_prob = log_probs[choice]
        choice_np = np.asarray(choice)
        log_prob_np = np.asarray(log_prob)

    return (choice_np, log_prob_np)
